# revision 16
# baseline (speedup 1.0000x reference)
"""Trainium2 Bass kernel for nn_Baseline_node2vec.

Computation (per pair e): logits[e] = relu(concat(embs[i_e], embs[j_e]) @ W1 + b1) @ W2 + b2

Strategy (per the sharding hint: "shard node_indices/gathered rows across M
devices, replicate the small MLP weights"): the host shards the E=1M pairs
across the 8 cores and ships each core its slice of the *gathered rows*,
pre-transposed to feature-on-partition layout xT=[256, E_pad] f16 (the gather
is pure indexing; every FLOP of the reference - W1, b1, relu, W2, b2 - runs
on device). The device streams xT in 2048-pair blocks over HWDGE DMA
(contiguous 4KB/partition segments, ~full HBM bandwidth) and runs a clean
3-engine pipeline per 512-pair chunk:
  - PE: hT = W1.T @ xT as 4 accumulating f16 matmuls (2 hid-halves x 2
    input-halves), then 2 accumulating W2 matmuls -> logitsT [2, 512] PSUM.
  - ACT: relu(h0 + b1) psum->sbuf f16 for hid-half 0.
  - DVE: fused scalar_tensor_tensor relu(h1 + b1) for hid-half 1.
  - logits move (+b2): alternating DVE tensor_scalar_add / ACT
    activation(Identity, bias=b2) per chunk to balance the engines.
Stage 2 (W2 matmuls + logits move) is emitted DEPTH=2 chunks behind stage 1
so the strict-FIFO PE queue never waits on relu output. Output is [2, E_pad]
channel-major; the host transposes back. Pair order is preserved end-to-end
(no reordering needed).
"""

import numpy as np

import concourse.bacc as bacc
import concourse.mybir as mybir
import concourse.tile as tile
from concourse import bass_utils
from concourse.bass_interp import get_hw_module

N_NODES = 100000
D = 128
HID = 256
E_TOTAL = 1000000
N_CORES = 8
E_CORE = E_TOTAL // N_CORES            # 125000
NB = 512                               # pairs per compute chunk
E_PAD = ((E_CORE + NB - 1) // NB) * NB  # 125440
G = 2048                               # pairs per DMA block

f32 = mybir.dt.float32
f16 = mybir.dt.float16
RELU = mybir.ActivationFunctionType.Relu
IDENT = mybir.ActivationFunctionType.Identity
ADD = mybir.AluOpType.add
MAXOP = mybir.AluOpType.max


def build_program(num_devices=N_CORES):
    nc = bacc.Bacc(
        "TRN2",
        target_bir_lowering=False,
        debug=False,
        enable_asserts=False,
        num_devices=num_devices,
    )

    xT = nc.dram_tensor("xT", [2 * D, E_PAD], f16, kind="ExternalInput").ap()
    w1 = nc.dram_tensor("w1", [2 * D, HID], f16, kind="ExternalInput").ap()
    b1v = nc.dram_tensor("b1v", [128, 2], f32, kind="ExternalInput").ap()
    w2 = nc.dram_tensor("w2", [HID, 2], f16, kind="ExternalInput").ap()
    b2v = nc.dram_tensor("b2v", [2, 1], f32, kind="ExternalInput").ap()
    outT = nc.dram_tensor("outT", [2, E_PAD], f32, kind="ExternalOutput").ap()

    n_blk, rem = divmod(E_PAD, G)
    sizes = [G] * n_blk + ([rem] if rem else [])

    with tile.TileContext(nc) as tc:
        with (
            tc.tile_pool(name="consts", bufs=1) as cpool,
            tc.tile_pool(name="xbuf", bufs=8) as xpool,
            tc.tile_pool(name="ht", bufs=8) as hpool,
            tc.tile_pool(name="ob", bufs=3) as opool,
            tc.tile_pool(name="ps_h", bufs=5, space="PSUM") as ps_h,
            tc.tile_pool(name="ps_l", bufs=3, space="PSUM") as ps_l,
        ):
            w1_sb = cpool.tile([128, 512], f16, name="w1_sb")
            nc.sync.dma_start(out=w1_sb[:, 0:256], in_=w1[0:128, :])
            nc.sync.dma_start(out=w1_sb[:, 256:512], in_=w1[128:256, :])
            w2_sb = cpool.tile([128, 4], f16, name="w2_sb")
            nc.sync.dma_start(out=w2_sb[:, 0:2], in_=w2[0:128, :])
            nc.sync.dma_start(out=w2_sb[:, 2:4], in_=w2[128:256, :])
            b1_sb = cpool.tile([128, 2], f32, name="b1_sb")
            nc.sync.dma_start(out=b1_sb[:], in_=b1v[:, :])
            b2_sb = cpool.tile([2, 1], f32, name="b2_sb")
            nc.sync.dma_start(out=b2_sb[:], in_=b2v[:, :])
            zeros = cpool.tile([128, NB], f16, name="zeros")
            nc.vector.memset(zeros[:], 0.0)

            # Software-pipelined emission: stage 1 (W1 matmuls + relu) runs
            # DEPTH chunks ahead of stage 2 (W2 matmuls + logits move), so the
            # strict-FIFO PE queue never stalls waiting for relu output.
            DEPTH = 2
            pend = []
            vtot = 0

            def stage2(ent):
                nonlocal vtot
                ht0, ht1, ob_e, sl_e, fin = ent
                lps = ps_l.tile([2, NB], f32, name="lps", tag="psl")
                nc.tensor.matmul(lps[:], w2_sb[:, 0:2], ht0[:],
                                 start=True, stop=False)
                nc.tensor.matmul(lps[:], w2_sb[:, 2:4], ht1[:],
                                 start=False, stop=True)
                if vtot % 2 == 0:
                    nc.vector.tensor_scalar_add(
                        out=ob_e[:, sl_e], in0=lps[:, :], scalar1=b2_sb[:])
                else:
                    nc.scalar.activation(ob_e[:, sl_e], lps[:, :], IDENT,
                                         bias=b2_sb[:], scale=1.0)
                vtot += 1
                if fin is not None:
                    out_off, out_sz, ob_fin = fin
                    nc.scalar.dma_start(
                        out=outT[:, out_off:out_off + out_sz],
                        in_=ob_fin[:, :out_sz],
                    )

            off = 0
            for sz in sizes:
                tl = xpool.tile([128, G], f16, name="tl", tag="tl")
                tr = xpool.tile([128, G], f16, name="tr", tag="tr")
                nc.sync.dma_start(out=tl[:, :sz], in_=xT[0:128, off:off + sz])
                nc.sync.dma_start(out=tr[:, :sz], in_=xT[128:256, off:off + sz])
                ob = opool.tile([2, G], f32, name="ob", tag="ob")
                n_v = sz // NB
                for v in range(n_v):
                    sl = slice(v * NB, (v + 1) * NB)
                    h0 = ps_h.tile([128, NB], f32, name="h0", tag="psh")
                    h1 = ps_h.tile([128, NB], f32, name="h1", tag="psh")
                    nc.tensor.matmul(h0[:], w1_sb[:, 0:128], tl[:, sl],
                                     start=True, stop=False)
                    nc.tensor.matmul(h0[:], w1_sb[:, 256:384], tr[:, sl],
                                     start=False, stop=True)
                    nc.tensor.matmul(h1[:], w1_sb[:, 128:256], tl[:, sl],
                                     start=True, stop=False)
                    nc.tensor.matmul(h1[:], w1_sb[:, 384:512], tr[:, sl],
                                     start=False, stop=True)
                    ht0 = hpool.tile([128, NB], f16, name="ht0", tag="ht")
                    ht1 = hpool.tile([128, NB], f16, name="ht1", tag="ht")
                    nc.scalar.activation(ht0[:], h0[:], RELU,
                                         bias=b1_sb[:, 0:1], scale=1.0)
                    nc.vector.scalar_tensor_tensor(
                        out=ht1[:], in0=h1[:], scalar=b1_sb[:, 1:2],
                        in1=zeros[:], op0=ADD, op1=MAXOP)
                    fin = (off, sz, ob) if v == n_v - 1 else None
                    pend.append((ht0, ht1, ob, sl, fin))
                    if len(pend) > DEPTH:
                        stage2(pend.pop(0))
                off += sz
            while pend:
                stage2(pend.pop(0))

    nc.compile()
    return nc


_CACHE = {}


def _get_program():
    if "nc" not in _CACHE:
        _CACHE["nc"] = build_program()
    return _CACHE["nc"]


def run_on_hw(nc, in_maps, trace=False, **kw):
    old = nc.m
    nc.m = get_hw_module(nc.m)
    try:
        return bass_utils.run_bass_kernel_spmd(
            nc, in_maps, core_ids=list(range(len(in_maps))), trace=trace, **kw
        )
    finally:
        nc.m = old


def make_in_maps(spatial_nodes_embs, node_indices, W1, b1, W2, b2):
    embs = np.ascontiguousarray(np.asarray(spatial_nodes_embs), dtype=np.float16)
    idx = np.asarray(node_indices).astype(np.int64)
    w1 = np.ascontiguousarray(np.asarray(W1), dtype=np.float16)
    b1 = np.asarray(b1, dtype=np.float32)
    w2 = np.ascontiguousarray(np.asarray(W2), dtype=np.float16)
    b2 = np.asarray(b2, dtype=np.float32)
    b1v = np.ascontiguousarray(b1.reshape(2, 128).T)
    b2v = np.ascontiguousarray(b2.reshape(2, 1))
    in_maps = []
    for c in range(N_CORES):
        ic = idx[c * E_CORE:(c + 1) * E_CORE]          # [E_CORE, 2]
        x = embs[ic.reshape(-1)].reshape(E_CORE, 2 * D)  # [E_CORE, 256]
        xT = np.zeros((2 * D, E_PAD), np.float16)
        xT[:, :E_CORE] = x.T
        in_maps.append({
            "xT": np.ascontiguousarray(xT), "w1": w1, "b1v": b1v,
            "w2": w2, "b2v": b2v,
        })
    return in_maps


def kernel(spatial_nodes_embs, node_indices, W1, b1, W2, b2):
    in_maps = make_in_maps(
        spatial_nodes_embs, node_indices, W1, b1, W2, b2)
    nc = _get_program()
    res = run_on_hw(nc, in_maps)
    outs = []
    for c in range(N_CORES):
        oT = res.results[c]["outT"]              # [2, E_PAD]
        outs.append(oT[:, :E_CORE].T)
    return np.ascontiguousarray(np.concatenate(outs, axis=0), dtype=np.float32)


# revision 17
# speedup vs baseline: 1.0530x; 1.0530x over previous
"""Trainium2 Bass kernel for nn_Baseline_node2vec.

Computation (per pair e): logits[e] = relu(concat(embs[i_e], embs[j_e]) @ W1 + b1) @ W2 + b2

Strategy (per the sharding hint: "shard node_indices/gathered rows across M
devices, replicate the small MLP weights"): the host shards the E=1M pairs
across the 8 cores and ships each core its slice of the *gathered rows*,
pre-transposed to feature-on-partition layout xT=[256, E_pad] f16 (the gather
is pure indexing; every FLOP of the reference - W1, b1, relu, W2, b2 - runs
on device). The device streams xT in 2048-pair blocks over HWDGE DMA
(contiguous 4KB/partition segments, ~full HBM bandwidth) and runs a clean
3-engine pipeline per 512-pair chunk:
  - PE: hT = W1.T @ xT as 4 accumulating f16 matmuls (2 hid-halves x 2
    input-halves), then 2 accumulating W2 matmuls -> logitsT [2, 512] PSUM.
  - ACT: relu(h0 + b1) psum->sbuf f16 for hid-half 0.
  - DVE: fused scalar_tensor_tensor relu(h1 + b1) for hid-half 1.
  - logits move (+b2): alternating DVE tensor_scalar_add / ACT
    activation(Identity, bias=b2) per chunk to balance the engines.
Stage 2 (W2 matmuls + logits move) is emitted DEPTH=2 chunks behind stage 1
so the strict-FIFO PE queue never waits on relu output. Output is [2, E_pad]
channel-major; the host transposes back. Pair order is preserved end-to-end
(no reordering needed).
"""

import numpy as np

import concourse.bacc as bacc
import concourse.mybir as mybir
import concourse.tile as tile
from concourse import bass_utils
from concourse.bass_interp import get_hw_module

N_NODES = 100000
D = 128
HID = 256
E_TOTAL = 1000000
N_CORES = 8
E_CORE = E_TOTAL // N_CORES            # 125000
NB = 512                               # pairs per compute chunk
E_PAD = ((E_CORE + NB - 1) // NB) * NB  # 125440
G = 2048                               # pairs per DMA block

f32 = mybir.dt.float32
f16 = mybir.dt.float16
RELU = mybir.ActivationFunctionType.Relu
IDENT = mybir.ActivationFunctionType.Identity
ADD = mybir.AluOpType.add
MAXOP = mybir.AluOpType.max


def build_program(num_devices=N_CORES):
    nc = bacc.Bacc(
        "TRN2",
        target_bir_lowering=False,
        debug=False,
        enable_asserts=False,
        num_devices=num_devices,
    )

    xT = nc.dram_tensor("xT", [2 * D, E_PAD], f16, kind="ExternalInput").ap()
    w1 = nc.dram_tensor("w1", [2 * D, HID], f16, kind="ExternalInput").ap()
    b1v = nc.dram_tensor("b1v", [128, 2], f32, kind="ExternalInput").ap()
    w2 = nc.dram_tensor("w2", [HID, 2], f16, kind="ExternalInput").ap()
    b2v = nc.dram_tensor("b2v", [2, 1], f32, kind="ExternalInput").ap()
    outT = nc.dram_tensor("outT", [2, E_PAD], f32, kind="ExternalOutput").ap()

    n_blk, rem = divmod(E_PAD, G)
    sizes = [G] * n_blk + ([rem] if rem else [])

    with tile.TileContext(nc) as tc:
        with (
            tc.tile_pool(name="consts", bufs=1) as cpool,
            tc.tile_pool(name="xbuf", bufs=8) as xpool,
            tc.tile_pool(name="ht", bufs=10) as hpool,
            tc.tile_pool(name="ob", bufs=3) as opool,
            tc.tile_pool(name="ps_h", bufs=5, space="PSUM") as ps_h,
            tc.tile_pool(name="ps_l", bufs=3, space="PSUM") as ps_l,
        ):
            w1_sb = cpool.tile([128, 512], f16, name="w1_sb")
            nc.sync.dma_start(out=w1_sb[:, 0:256], in_=w1[0:128, :])
            nc.sync.dma_start(out=w1_sb[:, 256:512], in_=w1[128:256, :])
            w2_sb = cpool.tile([128, 4], f16, name="w2_sb")
            nc.sync.dma_start(out=w2_sb[:, 0:2], in_=w2[0:128, :])
            nc.sync.dma_start(out=w2_sb[:, 2:4], in_=w2[128:256, :])
            b1_sb = cpool.tile([128, 2], f32, name="b1_sb")
            nc.sync.dma_start(out=b1_sb[:], in_=b1v[:, :])
            b2_sb = cpool.tile([2, 1], f32, name="b2_sb")
            nc.sync.dma_start(out=b2_sb[:], in_=b2v[:, :])
            zeros = cpool.tile([128, NB], f16, name="zeros")
            nc.vector.memset(zeros[:], 0.0)

            # Software-pipelined emission: stage 1 (W1 matmuls + relu) runs
            # DEPTH chunks ahead of stage 2 (W2 matmuls + logits move), so the
            # strict-FIFO PE queue never stalls waiting for relu output.
            DEPTH = 3
            pend = []
            vtot = 0

            def stage2(ent):
                nonlocal vtot
                ht0, ht1, ob_e, sl_e, fin = ent
                lps = ps_l.tile([2, NB], f32, name="lps", tag="psl")
                nc.tensor.matmul(lps[:], w2_sb[:, 0:2], ht0[:],
                                 start=True, stop=False)
                nc.tensor.matmul(lps[:], w2_sb[:, 2:4], ht1[:],
                                 start=False, stop=True)
                if vtot % 2 == 0:
                    nc.vector.tensor_scalar_add(
                        out=ob_e[:, sl_e], in0=lps[:, :], scalar1=b2_sb[:])
                else:
                    nc.scalar.activation(ob_e[:, sl_e], lps[:, :], IDENT,
                                         bias=b2_sb[:], scale=1.0)
                vtot += 1
                if fin is not None:
                    out_off, out_sz, ob_fin = fin
                    nc.scalar.dma_start(
                        out=outT[:, out_off:out_off + out_sz],
                        in_=ob_fin[:, :out_sz],
                    )

            off = 0
            for sz in sizes:
                tl = xpool.tile([128, G], f16, name="tl", tag="tl")
                tr = xpool.tile([128, G], f16, name="tr", tag="tr")
                nc.sync.dma_start(out=tl[:, :sz], in_=xT[0:128, off:off + sz])
                nc.sync.dma_start(out=tr[:, :sz], in_=xT[128:256, off:off + sz])
                ob = opool.tile([2, G], f32, name="ob", tag="ob")
                n_v = sz // NB
                for v in range(n_v):
                    sl = slice(v * NB, (v + 1) * NB)
                    h0 = ps_h.tile([128, NB], f32, name="h0", tag="psh")
                    h1 = ps_h.tile([128, NB], f32, name="h1", tag="psh")
                    nc.tensor.matmul(h0[:], w1_sb[:, 0:128], tl[:, sl],
                                     start=True, stop=False)
                    nc.tensor.matmul(h0[:], w1_sb[:, 256:384], tr[:, sl],
                                     start=False, stop=True)
                    nc.tensor.matmul(h1[:], w1_sb[:, 128:256], tl[:, sl],
                                     start=True, stop=False)
                    nc.tensor.matmul(h1[:], w1_sb[:, 384:512], tr[:, sl],
                                     start=False, stop=True)
                    ht0 = hpool.tile([128, NB], f16, name="ht0", tag="ht")
                    ht1 = hpool.tile([128, NB], f16, name="ht1", tag="ht")
                    nc.scalar.activation(ht0[:], h0[:], RELU,
                                         bias=b1_sb[:, 0:1], scale=1.0)
                    nc.vector.scalar_tensor_tensor(
                        out=ht1[:], in0=h1[:], scalar=b1_sb[:, 1:2],
                        in1=zeros[:], op0=ADD, op1=MAXOP)
                    fin = (off, sz, ob) if v == n_v - 1 else None
                    pend.append((ht0, ht1, ob, sl, fin))
                    if len(pend) > DEPTH:
                        stage2(pend.pop(0))
                off += sz
            while pend:
                stage2(pend.pop(0))

    nc.compile()
    return nc


_CACHE = {}


def _get_program():
    if "nc" not in _CACHE:
        _CACHE["nc"] = build_program()
    return _CACHE["nc"]


def run_on_hw(nc, in_maps, trace=False, **kw):
    old = nc.m
    nc.m = get_hw_module(nc.m)
    try:
        return bass_utils.run_bass_kernel_spmd(
            nc, in_maps, core_ids=list(range(len(in_maps))), trace=trace, **kw
        )
    finally:
        nc.m = old


def make_in_maps(spatial_nodes_embs, node_indices, W1, b1, W2, b2):
    embs = np.ascontiguousarray(np.asarray(spatial_nodes_embs), dtype=np.float16)
    idx = np.asarray(node_indices).astype(np.int64)
    w1 = np.ascontiguousarray(np.asarray(W1), dtype=np.float16)
    b1 = np.asarray(b1, dtype=np.float32)
    w2 = np.ascontiguousarray(np.asarray(W2), dtype=np.float16)
    b2 = np.asarray(b2, dtype=np.float32)
    b1v = np.ascontiguousarray(b1.reshape(2, 128).T)
    b2v = np.ascontiguousarray(b2.reshape(2, 1))
    in_maps = []
    for c in range(N_CORES):
        ic = idx[c * E_CORE:(c + 1) * E_CORE]          # [E_CORE, 2]
        x = embs[ic.reshape(-1)].reshape(E_CORE, 2 * D)  # [E_CORE, 256]
        xT = np.zeros((2 * D, E_PAD), np.float16)
        xT[:, :E_CORE] = x.T
        in_maps.append({
            "xT": np.ascontiguousarray(xT), "w1": w1, "b1v": b1v,
            "w2": w2, "b2v": b2v,
        })
    return in_maps


def kernel(spatial_nodes_embs, node_indices, W1, b1, W2, b2):
    in_maps = make_in_maps(
        spatial_nodes_embs, node_indices, W1, b1, W2, b2)
    nc = _get_program()
    res = run_on_hw(nc, in_maps)
    outs = []
    for c in range(N_CORES):
        oT = res.results[c]["outT"]              # [2, E_PAD]
        outs.append(oT[:, :E_CORE].T)
    return np.ascontiguousarray(np.concatenate(outs, axis=0), dtype=np.float32)
